# revision 4
# baseline (speedup 1.0000x reference)
"""Cross-attention kernel for Trainium2, 8 NeuronCores.

Sharding: core = (batch b in 0..3) x (head-group hg in 0..1).
Each core computes, for its batch and its 8 heads (512 of the 1024 H cols):
    qT = (Wq_h^T @ query[b]^T)        [512, SQ]   (+bq per-partition)
    kT = (Wk_h^T @ key_value[b]^T)    [512, SKV]  (+bk per-partition)
    v  = key_value[b] @ Wv_h          [SKV, 512]  (stored per kv-tile with a
                                                   ones-column per head: [128, 8*65])
    per head h, per q-chunk:
        scoresT = kT_h^T-slice matmuls -> [kv 128, q]  (PSUM)
        expT    = exp(scoresT / 8)                      (ACT, PSUM->SBUF)
        outT_h  = v_aug_h^T @ expT -> [65, q] PSUM accum over kv tiles;
                  row 64 = softmax denominator (ones column)
        normalize: recip(row64) -> PE broadcast to [65,q] -> DVE multiply
    out_partial = (attn_outT as lhsT) @ Wo_h  -> [SQ, 1024]  natural, DMA out.
Host sums the two head-group partials per batch and adds bv@Wo + bo.

All matmuls are fp32 (PE streams 1 col/cycle regardless of dtype).
Softmax skips max-subtraction: |scores| <= ~25 for this problem's scale
(weights ~N(0, 0.02^2), so exp is far from fp32 overflow).
"""

import numpy as np

import concourse.bass as bass
import concourse.mybir as mybir
import concourse.tile as tile
from concourse import bacc
from concourse import bass_utils

FP32 = mybir.dt.float32
P = 128

B, SQ, SKV = 4, 2048, 2048
D, H, NH, HD = 1024, 1024, 16, 64
HC = 512          # H columns per core (8 heads)
NHC = 8           # heads per core
VW = HD + 1       # v columns per head incl. ones column


def build_core_program(sq=SQ, skv=SKV, n_devices=8):
    nc = bacc.Bacc(
        "TRN2",
        target_bir_lowering=False,
        debug=False,
        enable_asserts=False,
        num_devices=n_devices,
    )

    xqT = nc.dram_tensor("xqT", (D, sq), FP32, kind="ExternalInput").ap()
    xkT = nc.dram_tensor("xkT", (D, skv), FP32, kind="ExternalInput").ap()
    wq = nc.dram_tensor("wq", (D, HC), FP32, kind="ExternalInput").ap()
    wk = nc.dram_tensor("wk", (D, HC), FP32, kind="ExternalInput").ap()
    wv = nc.dram_tensor("wv", (D, HC), FP32, kind="ExternalInput").ap()
    wo = nc.dram_tensor("wo", (HC, D), FP32, kind="ExternalInput").ap()
    bq = nc.dram_tensor("bq", (HC, 1), FP32, kind="ExternalInput").ap()
    bk = nc.dram_tensor("bk", (HC, 1), FP32, kind="ExternalInput").ap()
    out = nc.dram_tensor("out", (sq, D), FP32, kind="ExternalOutput").ap()

    ND = D // P            # 8 contraction chunks for projections
    NI = HC // P           # 4 Hc tiles
    NQT = sq // P          # q tiles
    NKT = skv // P         # kv tiles
    PC = min(512, sq)      # projection q/kv chunk
    NPCQ = sq // PC
    PCK = min(512, skv)
    NPCK = skv // PCK
    QC = min(1024, sq)     # attention q chunk (2 PSUM banks)
    NQC = sq // QC
    SUB = 512              # matmul moving-operand max for fp32

    EXP = mybir.ActivationFunctionType.Exp

    with tile.TileContext(nc) as tc:
        with tc.tile_pool(name="persist", bufs=1) as persist:
            qT = [persist.tile([P, sq], FP32, tag=f"qT{i}", name=f"qT{i}") for i in range(NI)]
            kT = [persist.tile([P, skv], FP32, tag=f"kT{i}", name=f"kT{i}") for i in range(NI)]
            vaug = [persist.tile([P, NHC * VW], FP32, tag=f"v{t}", name=f"v{t}") for t in range(NKT)]
            bqs = persist.tile([P, NI], FP32, tag="bqs")
            bks = persist.tile([P, NI], FP32, tag="bks")
            ones65 = persist.tile([1, VW], FP32, tag="ones65")

            nc.vector.memset(ones65[:], 1.0)
            for i in range(NI):
                nc.sync.dma_start(out=bqs[:, i : i + 1], in_=bq[i * P : (i + 1) * P, :])
                nc.sync.dma_start(out=bks[:, i : i + 1], in_=bk[i * P : (i + 1) * P, :])

            # ---------------- projections ----------------
            with (
                tc.tile_pool(name="wts", bufs=1) as wts,
                tc.tile_pool(name="xs", bufs=12) as xs,
                tc.tile_pool(name="ppsum", bufs=4, space=bass.MemorySpace.PSUM) as ppsum,
                tc.tile_pool(name="vpsum", bufs=2, space=bass.MemorySpace.PSUM) as vpsum,
            ):
                wq_sb = [wts.tile([P, HC], FP32, tag=f"wq{d}", name=f"wq{d}") for d in range(ND)]
                wk_sb = [wts.tile([P, HC], FP32, tag=f"wk{d}", name=f"wk{d}") for d in range(ND)]
                wv_sb = [wts.tile([P, HC], FP32, tag=f"wv{d}", name=f"wv{d}") for d in range(ND)]
                for d in range(ND):
                    nc.sync.dma_start(out=wq_sb[d][:], in_=wq[d * P : (d + 1) * P, :])
                    nc.sync.dma_start(out=wk_sb[d][:], in_=wk[d * P : (d + 1) * P, :])
                    nc.sync.dma_start(out=wv_sb[d][:], in_=wv[d * P : (d + 1) * P, :])

                # qT projection: out[Hc-tile, q-chunk] accum over d
                for c in range(NPCQ):
                    xq_c = []
                    for d in range(ND):
                        t = xs.tile([P, PC], FP32, tag="x", name="xq")
                        nc.sync.dma_start(
                            out=t[:], in_=xqT[d * P : (d + 1) * P, c * PC : (c + 1) * PC]
                        )
                        xq_c.append(t)
                    for i in range(NI):
                        ps = ppsum.tile([P, PC], FP32, tag="pp")
                        for d in range(ND):
                            nc.tensor.matmul(
                                ps[:],
                                wq_sb[d][:, i * P : (i + 1) * P],
                                xq_c[d][:],
                                start=(d == 0),
                                stop=(d == ND - 1),
                            )
                        nc.vector.tensor_scalar_add(
                            out=qT[i][:, c * PC : (c + 1) * PC],
                            in0=ps[:],
                            scalar1=bqs[:, i : i + 1],
                        )

                # kT + v projections share the xkT chunk stream
                for c in range(NPCK):
                    xk_c = []
                    for d in range(ND):
                        t = xs.tile([P, PCK], FP32, tag="x", name="xk")
                        nc.sync.dma_start(
                            out=t[:], in_=xkT[d * P : (d + 1) * P, c * PCK : (c + 1) * PCK]
                        )
                        xk_c.append(t)
                    for i in range(NI):
                        ps = ppsum.tile([P, PCK], FP32, tag="pp")
                        for d in range(ND):
                            nc.tensor.matmul(
                                ps[:],
                                wk_sb[d][:, i * P : (i + 1) * P],
                                xk_c[d][:],
                                start=(d == 0),
                                stop=(d == ND - 1),
                            )
                        nc.vector.tensor_scalar_add(
                            out=kT[i][:, c * PCK : (c + 1) * PCK],
                            in0=ps[:],
                            scalar1=bks[:, i : i + 1],
                        )
                    # v: natural orientation [kv-tile, Hc] accum over d
                    for tt in range(PCK // P):
                        kvt = c * (PCK // P) + tt
                        ps = vpsum.tile([P, HC], FP32, tag="pv")
                        for d in range(ND):
                            nc.tensor.matmul(
                                ps[:],
                                xk_c[d][:, tt * P : (tt + 1) * P],
                                wv_sb[d][:],
                                start=(d == 0),
                                stop=(d == ND - 1),
                            )
                        nc.gpsimd.memset(vaug[kvt][:], 1.0)
                        for h in range(NHC):
                            nc.vector.tensor_copy(
                                out=vaug[kvt][:, h * VW : h * VW + HD],
                                in_=ps[:, h * HD : (h + 1) * HD],
                            )

            # ---------------- attention + output projection ----------------
            with (
                tc.tile_pool(name="wop", bufs=1) as wop,
                tc.tile_pool(name="otp", bufs=1) as otp,
                tc.tile_pool(name="esb", bufs=4) as esb,
                tc.tile_pool(name="smalls", bufs=4) as smalls,
            ):
                wo_sb = [wop.tile([P, D], FP32, tag=f"wo{j}", name=f"wo{j}") for j in range(NI)]
                for j in range(NI):
                    nc.sync.dma_start(out=wo_sb[j][:], in_=wo[j * P : (j + 1) * P, :])
                outT = [otp.tile([P, sq], FP32, tag=f"oT{i}", name=f"oT{i}") for i in range(NI)]

                with (
                    tc.tile_pool(name="scps", bufs=2, space=bass.MemorySpace.PSUM) as scps,
                    tc.tile_pool(name="ovps", bufs=2, space=bass.MemorySpace.PSUM) as ovps,
                ):
                    for h in range(NHC):
                        i, r = h // 2, (h % 2) * HD
                        for c in range(NQC):
                            ovt = ovps.tile([VW, QC], FP32, tag="ov")
                            for t in range(NKT):
                                sc = scps.tile([P, QC], FP32, tag="sc")
                                for s in range(0, QC, SUB):
                                    w = min(SUB, QC - s)
                                    nc.tensor.matmul(
                                        sc[:, s : s + w],
                                        kT[i][r : r + HD, t * P : (t + 1) * P],
                                        qT[i][r : r + HD, c * QC + s : c * QC + s + w],
                                        start=True,
                                        stop=True,
                                    )
                                et = esb.tile([P, QC], FP32, tag="e")
                                nc.scalar.activation(et[:], sc[:], EXP, scale=0.125)
                                for s in range(0, QC, SUB):
                                    w = min(SUB, QC - s)
                                    nc.tensor.matmul(
                                        ovt[:, s : s + w],
                                        vaug[t][:, h * VW : (h + 1) * VW],
                                        et[:, s : s + w],
                                        start=(t == 0),
                                        stop=(t == NKT - 1),
                                    )
                            # normalize: row 64 of ovt is the denominator
                            rec = smalls.tile([1, QC], FP32, tag="rec")
                            nc.vector.reciprocal(rec[:], ovt[HD : HD + 1, :])
                            bc = scps.tile([VW, QC], FP32, tag="sc")
                            for s in range(0, QC, SUB):
                                w = min(SUB, QC - s)
                                nc.tensor.matmul(
                                    bc[:, s : s + w],
                                    ones65[:],
                                    rec[:, s : s + w],
                                    start=True,
                                    stop=True,
                                )
                            # only one PSUM operand allowed per DVE op:
                            # stage the broadcast in SBUF, then multiply
                            bcs = esb.tile([HD, QC], FP32, tag="bcs", name="bcs")
                            nc.vector.tensor_copy(out=bcs[:], in_=bc[0:HD, :])
                            nc.vector.tensor_mul(
                                out=outT[i][r : r + HD, c * QC : (c + 1) * QC],
                                in0=ovt[0:HD, :],
                                in1=bcs[:],
                            )

                # output projection: natural [q-tile, Dq-chunk]
                with (
                    tc.tile_pool(name="opps", bufs=4, space=bass.MemorySpace.PSUM) as opps,
                    tc.tile_pool(name="ost", bufs=4) as ost,
                ):
                    for m in range(NQT):
                        for n in range(D // 512):
                            ps = opps.tile([P, 512], FP32, tag="op")
                            for j in range(NI):
                                nc.tensor.matmul(
                                    ps[:],
                                    outT[j][:, m * P : (m + 1) * P],
                                    wo_sb[j][:, n * 512 : (n + 1) * 512],
                                    start=(j == 0),
                                    stop=(j == NI - 1),
                                )
                            ot = ost.tile([P, 512], FP32, tag="ot")
                            nc.vector.tensor_copy(out=ot[:], in_=ps[:])
                            nc.sync.dma_start(
                                out=out[m * P : (m + 1) * P, n * 512 : (n + 1) * 512],
                                in_=ot[:],
                            )

    nc.compile()
    return nc


_CACHED_NC = None


def _get_nc():
    global _CACHED_NC
    if _CACHED_NC is None:
        _CACHED_NC = build_core_program()
    return _CACHED_NC


def make_in_maps(query, key_value, Wq, bq, Wk, bk, Wv, bv, Wo, bo):
    query = np.asarray(query, np.float32)
    key_value = np.asarray(key_value, np.float32)
    Wq = np.asarray(Wq, np.float32)
    Wk = np.asarray(Wk, np.float32)
    Wv = np.asarray(Wv, np.float32)
    Wo = np.asarray(Wo, np.float32)
    bq = np.asarray(bq, np.float32)
    bk = np.asarray(bk, np.float32)

    in_maps = []
    for core in range(8):
        b, hg = core // 2, core % 2
        hs = hg * HC
        in_maps.append(
            {
                "xqT": np.ascontiguousarray(query[b].T),
                "xkT": np.ascontiguousarray(key_value[b].T),
                "wq": np.ascontiguousarray(Wq[:, hs : hs + HC]),
                "wk": np.ascontiguousarray(Wk[:, hs : hs + HC]),
                "wv": np.ascontiguousarray(Wv[:, hs : hs + HC]),
                "wo": np.ascontiguousarray(Wo[hs : hs + HC, :]),
                "bq": np.ascontiguousarray(bq[hs : hs + HC, None]),
                "bk": np.ascontiguousarray(bk[hs : hs + HC, None]),
            }
        )
    return in_maps


def _install_profiling():
    """Reconstruct the NTFF profile hook this container's boot skipped.

    bass_utils' axon trace path wants antenv.axon_hooks (absent here);
    inject a stub module and register the ctypes-based hook from
    trn_agent_boot. Also keep artifacts local (no bucket in container).
    """
    import sys
    import types

    if "antenv.axon_hooks" in sys.modules:
        return
    import antenv  # noqa: F401

    mod = types.ModuleType("antenv.axon_hooks")
    mod._hook = None

    def set_axon_ntff_profile_hook(h):
        mod._hook = h

    def get_axon_ntff_profile_hook():
        return mod._hook

    mod.set_axon_ntff_profile_hook = set_axon_ntff_profile_hook
    mod.get_axon_ntff_profile_hook = get_axon_ntff_profile_hook
    sys.modules["antenv.axon_hooks"] = mod

    from trn_agent_boot.trn_boot import _ntff_profile_via_ctypes

    hook = _ntff_profile_via_ctypes("/opt/axon/libaxon_pjrt.so")
    if hook is not None:
        set_axon_ntff_profile_hook(hook)

    bass_utils.upload_artifacts = lambda tmpdir: tmpdir


def run_device(inputs, trace=False, **kw):
    if trace:
        _install_profiling()
    nc = _get_nc()
    in_maps = make_in_maps(**inputs)
    res = bass_utils.run_bass_kernel_spmd(
        nc, in_maps, list(range(8)), trace=trace, **kw
    )
    return res


def assemble_output(results, Wv_bias_term):
    out = np.zeros((B, SQ, D), np.float32)
    for core in range(8):
        b = core // 2
        out[b] += results[core]["out"]
    out += Wv_bias_term
    return out


def kernel(**inputs):
    res = run_device(inputs)
    bv = np.asarray(inputs["bv"], np.float32)
    bo = np.asarray(inputs["bo"], np.float32)
    Wo = np.asarray(inputs["Wo"], np.float32)
    # attn rows sum to 1, so the bv shift passes straight through attn@v;
    # bv@Wo + bo is added once on the host.
    bias_term = bv @ Wo + bo
    return assemble_output(res.results, bias_term)
